# revision 9
# baseline (speedup 1.0000x reference)
"""nn_CNF Trainium2 Bass kernel — 8-core data-parallel.

Math (per batch row b of z (B, 32)):
    h      = tanh(z @ Wt.T + Bt)            (B, 64)
    dz_dt  = (h @ Ut) / 64                  (B, 32)
    dlogp  = (h^2 @ wu - sum(wu)) / 64      (B, 1)   [= -(1-h^2)@wu/64]
Wt/Ut/Bt come from a tiny RBF hypernetwork of t (computed on host),
wu = sum(Wt*Ut, axis=1).

Device layout (per core, 131072 rows, all matmuls bf16; mm1 uses an exact
hi/lo bf16 split of z and Wt — 3 accumulation passes, error ~1e-5):
  zT (128, 32768): partition 32q+d = z[q*32768 + n, d], q = batch quarter.
  64 superblocks of 512 columns (2048 rows each):
    mm1: 3 passes x 4 concurrent row/col-tiled K=32 matmuls -> PSUM (128,1024)
    ACT tanh + bias -> S_h (128, 1024) bf16
    DVE square      -> S_g (128, 1024) bf16
    mm2h: 2 block-diag K=128 M=64 matmuls -> dz.T packed (128, 512) PSUM
          ACT Copy evac -> SBUF (batched 4 sb) -> DMA out
    mm2g: 2 M=2 matmuls -> dlogp strips, partition-rotated over sb pairs,
          DVE evac once per 2 sb -> 4 small DMAs
Host unpacks dzT and applies the dlogp constant.
"""
import numpy as np

NCORES = 8
BATCH = 1048576
D = 32
WIDTH = 64
PER_CORE = BATCH // NCORES      # 131072
QUARTER = PER_CORE // 4         # 32768
SB = 512                        # superblock columns
NSB = QUARTER // SB             # 64
ZCHUNK = 2048                   # z-in DMA columns (4 superblocks, 512KB/tensor)

_CACHE = {}


def _hypernet(t, centres, log_sigmas, lin_w, lin_b):
    # matches reference._rbf_linear in float32
    d = np.abs(t - centres[:, 0]) / np.exp(log_sigmas)
    phi = np.exp(-(d * d)).astype(np.float32)
    return phi @ lin_w.T + lin_b


def _split_bf16(x):
    """x (f32) -> (hi, lo) bf16 with hi + lo ~= x (error ~2^-17 rel)."""
    import ml_dtypes
    hi = x.astype(ml_dtypes.bfloat16)
    lo = (x - hi.astype(np.float32)).astype(ml_dtypes.bfloat16)
    return hi, lo


def _build(reps=1):
    import concourse.bass as bass
    import concourse.tile as tile
    from concourse import bacc, mybir
    from contextlib import ExitStack

    f32 = mybir.dt.float32
    bf16 = mybir.dt.bfloat16
    AF = mybir.ActivationFunctionType

    nc = bacc.Bacc("TRN2", target_bir_lowering=False, debug=False,
                   enable_asserts=True, num_devices=NCORES)

    zhi_d = nc.dram_tensor("z_hi", [128, QUARTER], bf16, kind="ExternalInput").ap()
    zlo_d = nc.dram_tensor("z_lo", [128, QUARTER], bf16, kind="ExternalInput").ap()
    whi_d = nc.dram_tensor("wtT_hi", [128, WIDTH], bf16, kind="ExternalInput").ap()
    wlo_d = nc.dram_tensor("wtT_lo", [128, WIDTH], bf16, kind="ExternalInput").ap()
    l2_d = nc.dram_tensor("lhsT2", [128, 128], bf16, kind="ExternalInput").ap()
    lg_d = nc.dram_tensor("lhsTg", [128, 2], bf16, kind="ExternalInput").ap()
    b_d = nc.dram_tensor("bias", [128, 1], f32, kind="ExternalInput").ap()

    dzT_d = nc.dram_tensor("dzT", [128, QUARTER], f32, kind="ExternalOutput").ap()
    dlp_d = nc.dram_tensor("dlp", [4, QUARTER], f32, kind="ExternalOutput").ap()

    with tile.TileContext(nc) as tc, ExitStack() as ctx:
        const = ctx.enter_context(tc.tile_pool(name="const", bufs=1))
        zin = ctx.enter_context(tc.tile_pool(name="zin", bufs=2))
        shp = ctx.enter_context(tc.tile_pool(name="sh", bufs=2))
        sgp = ctx.enter_context(tc.tile_pool(name="sg", bufs=2))
        dzp = ctx.enter_context(tc.tile_pool(name="dz", bufs=2))
        dlpp = ctx.enter_context(tc.tile_pool(name="dlp", bufs=2))
        # PSUM: p1 2 banks x2 + p2 1 bank x2 + pg 1 bank x2 = 8 banks
        p1p = ctx.enter_context(tc.tile_pool(name="p1", bufs=2, space="PSUM"))
        p2p = ctx.enter_context(tc.tile_pool(name="p2", bufs=2, space="PSUM"))
        pgp = ctx.enter_context(tc.tile_pool(name="pg", bufs=2, space="PSUM"))

        whi_t = const.tile([128, WIDTH], bf16)
        nc.sync.dma_start(whi_t[:], whi_d[:])
        wlo_t = const.tile([128, WIDTH], bf16)
        nc.sync.dma_start(wlo_t[:], wlo_d[:])
        l2_t = const.tile([128, 128], bf16)
        nc.sync.dma_start(l2_t[:], l2_d[:])
        lg_t = const.tile([128, 2], bf16)
        nc.sync.dma_start(lg_t[:], lg_d[:])
        b_t = const.tile([128, 1], f32)
        nc.sync.dma_start(b_t[:], b_d[:])

        zhi_t = zlo_t = None
        dzbig = None
        pg = None

        def body():
            for m in range(NSB):
                emit_sb(m)

        def emit_sb(m):
            nonlocal zhi_t, zlo_t, dzbig, pg
            if m % 4 == 0:
                zhi_t = zin.tile([128, ZCHUNK], bf16, tag="zhi")
                nc.sync.dma_start(zhi_t[:], zhi_d[:, m * SB:m * SB + ZCHUNK])
                zlo_t = zin.tile([128, ZCHUNK], bf16, tag="zlo")
                nc.sync.dma_start(zlo_t[:], zlo_d[:, m * SB:m * SB + ZCHUNK])
            c4 = (m % 4) * SB
            zhc = zhi_t[:, c4:c4 + SB]
            zlc = zlo_t[:, c4:c4 + SB]

            # mm1: 3 hi/lo passes x 4 concurrent row/col tiles
            p1 = p1p.tile([128, 2 * SB], f32)
            for ip, (wt, zt) in enumerate(((whi_t, zhc), (whi_t, zlc),
                                           (wlo_t, zhc))):
                for q in range(4):
                    half, bank = q % 2, q // 2
                    nc.tensor.matmul(
                        p1[half * 64:(half + 1) * 64, bank * SB:(bank + 1) * SB],
                        wt[32 * q:32 * (q + 1), :],
                        zt[32 * q:32 * (q + 1), :],
                        start=(ip == 0), stop=(ip == 2),
                        tile_position=(32 * q, 64 * half),
                    )

            sh = shp.tile([128, 2 * SB], bf16)
            nc.scalar.activation(sh[:], p1[:], AF.Tanh, bias=b_t[:], scale=1.0)
            sg = sgp.tile([128, 2 * SB], bf16)
            nc.vector.tensor_mul(sg[:], sh[:], sh[:])

            # mm2h: dz.T packed (partition 32q+d)
            p2 = p2p.tile([128, SB], f32)
            nc.tensor.matmul(p2[0:64, :], l2_t[:, 0:64], sh[:, 0:SB],
                             start=True, stop=True, tile_position=(0, 0))
            nc.tensor.matmul(p2[64:128, :], l2_t[:, 64:128], sh[:, SB:2 * SB],
                             start=True, stop=True, tile_position=(0, 64))

            if m % 4 == 0:
                dzbig = dzp.tile([128, ZCHUNK], f32)
            nc.scalar.activation(dzbig[:, c4:c4 + SB], p2[:], AF.Copy)
            if m % 4 == 3:
                nc.sync.dma_start(dzT_d[:, (m - 3) * SB:(m + 1) * SB], dzbig[:])

            # mm2g: dlogp strips; partition offset rotates over sb pairs
            off = 64 * (m % 2)
            if m % 2 == 0:
                pg = pgp.tile([128, SB], f32)
            nc.tensor.matmul(pg[off + 0:off + 2, :], lg_t[:], sg[:, 0:SB],
                             start=True, stop=True, tile_position=(0, off))
            nc.tensor.matmul(pg[off + 32:off + 34, :], lg_t[:], sg[:, SB:2 * SB],
                             start=True, stop=True, tile_position=(0, off + 32))
            if m % 2 == 1:
                dst = dlpp.tile([128, SB], f32)
                nc.vector.tensor_copy(dst[0:98, :], pg[0:98, :])
                nc.sync.dma_start(dlp_d[0:2, (m - 1) * SB:m * SB], dst[0:2, :])
                nc.sync.dma_start(dlp_d[2:4, (m - 1) * SB:m * SB], dst[32:34, :])
                nc.sync.dma_start(dlp_d[0:2, m * SB:(m + 1) * SB], dst[64:66, :])
                nc.sync.dma_start(dlp_d[2:4, m * SB:(m + 1) * SB], dst[96:98, :])

        if reps == 1:
            body()
        else:
            # timing-only variant: hardware loop repeating identical work
            with tc.For_i(0, reps, 1):
                body()

    nc.compile()
    return nc


def _get_nc(reps=1):
    key = f"nc{reps}"
    if key not in _CACHE:
        _CACHE[key] = _build(reps)
    return _CACHE[key]


def _prep_inputs(t, z,
                 W_centres, W_log_sigmas, W_lin_w, W_lin_b,
                 U_centres, U_log_sigmas, U_lin_w, U_lin_b,
                 B_centres, B_log_sigmas, B_lin_w, B_lin_b):
    t = np.asarray(t, np.float32)
    Wt = _hypernet(t, np.asarray(W_centres, np.float32),
                   np.asarray(W_log_sigmas, np.float32),
                   np.asarray(W_lin_w, np.float32),
                   np.asarray(W_lin_b, np.float32)).reshape(WIDTH, D)
    Ut = _hypernet(t, np.asarray(U_centres, np.float32),
                   np.asarray(U_log_sigmas, np.float32),
                   np.asarray(U_lin_w, np.float32),
                   np.asarray(U_lin_b, np.float32)).reshape(WIDTH, D)
    Bt = _hypernet(t, np.asarray(B_centres, np.float32),
                   np.asarray(B_log_sigmas, np.float32),
                   np.asarray(B_lin_w, np.float32),
                   np.asarray(B_lin_b, np.float32))
    wu = np.sum(Wt * Ut, axis=1)

    U64 = (Ut / np.float32(WIDTH)).astype(np.float32)          # (64, 32)
    wu64 = (wu / np.float32(WIDTH)).astype(np.float32)         # (64,)

    import ml_dtypes
    bf = ml_dtypes.bfloat16

    wtT = np.tile(Wt.T, (4, 1)).astype(np.float32)             # (128, 64)
    wtT_hi, wtT_lo = _split_bf16(wtT)

    blk = np.zeros((128, 64), np.float32)
    blk[0:64, 0:32] = U64
    blk[64:128, 32:64] = U64
    lhsT2 = np.concatenate([blk, blk], axis=1).astype(bf)      # (128, 128)
    lhsTg = np.zeros((128, 2), np.float32)
    lhsTg[0:64, 0] = wu64
    lhsTg[64:128, 1] = wu64
    lhsTg = lhsTg.astype(bf)
    bias = np.concatenate([Bt, Bt]).reshape(128, 1).astype(np.float32)

    z = np.ascontiguousarray(np.asarray(z, np.float32))
    # per-core packed transpose: (8, 4, 32768, 32) -> (8, 4, 32, 32768)
    zT = z.reshape(NCORES, 4, QUARTER, D).transpose(0, 1, 3, 2) \
          .reshape(NCORES, 128, QUARTER)
    z_hi, z_lo = _split_bf16(zT)
    z_hi = np.ascontiguousarray(z_hi)
    z_lo = np.ascontiguousarray(z_lo)

    in_maps = [
        dict(z_hi=z_hi[c], z_lo=z_lo[c], wtT_hi=wtT_hi, wtT_lo=wtT_lo,
             lhsT2=lhsT2, lhsTg=lhsTg, bias=bias)
        for c in range(NCORES)
    ]
    return in_maps, wu


def _postprocess(results, wu):
    const = np.float32(np.sum(wu) / np.float32(WIDTH))
    dz_parts, dlp_parts = [], []
    for c in range(NCORES):
        dzT = results[c]["dzT"]                       # (128, 32768)
        dz = dzT.reshape(4, D, QUARTER).transpose(0, 2, 1).reshape(PER_CORE, D)
        dz_parts.append(dz)
        dlp = results[c]["dlp"].reshape(PER_CORE) - const
        dlp_parts.append(dlp)
    dz_dt = np.ascontiguousarray(np.concatenate(dz_parts, axis=0), dtype=np.float32)
    dlogp = np.concatenate(dlp_parts, axis=0).reshape(BATCH, 1).astype(np.float32)
    return dz_dt, dlogp


def kernel(t, z, logp_z, **params):
    from concourse.bass_utils import run_bass_kernel_spmd

    nc = _get_nc()
    in_maps, wu = _prep_inputs(t, z, **params)
    res = run_bass_kernel_spmd(nc, in_maps, core_ids=list(range(NCORES)))
    return _postprocess(res.results, wu)


# revision 15
# speedup vs baseline: 2.1443x; 2.1443x over previous
"""nn_CNF Trainium2 Bass kernel — 8-core data-parallel.

Math (per batch row b of z (B, 32)):
    h      = tanh(z @ Wt.T + Bt)            (B, 64)
    dz_dt  = (h @ Ut) / 64                  (B, 32)
    dlogp  = (h^2 @ wu - sum(wu)) / 64      (B, 1)   [= -(1-h^2)@wu/64]
Wt/Ut/Bt come from a tiny RBF hypernetwork of t (computed on host),
wu = sum(Wt*Ut, axis=1).

Device layout (per core, 131072 rows, all matmuls bf16; mm1 uses an exact
hi/lo bf16 split of z and Wt — 3 accumulation passes, error ~1e-5):
  zT (128, 32768): partition 32q+d = z[q*32768 + n, d], q = batch quarter.
  64 superblocks of 512 columns (2048 rows each):
    mm1: 3 passes x 4 concurrent row/col-tiled K=32 matmuls -> PSUM (128,1024)
    ACT tanh + bias -> S_h (128, 1024) bf16
    DVE square      -> S_g (128, 1024) bf16
    mm2h: 2 block-diag K=128 M=64 matmuls -> dz.T packed (128, 512) PSUM
          ACT Copy evac -> SBUF (batched 4 sb) -> DMA out
    mm2g: 2 M=2 matmuls -> dlogp strips, partition-rotated over sb pairs,
          DVE evac once per 2 sb -> 4 small DMAs
Host unpacks dzT and applies the dlogp constant.
"""
import numpy as np

NCORES = 8
BATCH = 1048576
D = 32
WIDTH = 64
PER_CORE = BATCH // NCORES      # 131072
QUARTER = PER_CORE // 4         # 32768
SB = 512                        # superblock columns
NSB = QUARTER // SB             # 64
ZCHUNK = 4096                   # z-in DMA columns (8 superblocks, 1MB/tensor)
DZCHUNK = 4096                  # dz-out staging columns (8 superblocks, 2MB)
DLPGRP = 16                     # superblocks per dlogp flush (4 DMAs each)
ACTCOLS = 320                   # dz evac columns copied by ACT (rest on DVE)

_CACHE = {}


def _hypernet(t, centres, log_sigmas, lin_w, lin_b):
    # matches reference._rbf_linear in float32
    d = np.abs(t - centres[:, 0]) / np.exp(log_sigmas)
    phi = np.exp(-(d * d)).astype(np.float32)
    return phi @ lin_w.T + lin_b


def _split_bf16(x):
    """x (f32) -> (hi, lo) bf16 with hi + lo ~= x (error ~2^-17 rel)."""
    import ml_dtypes
    hi = x.astype(ml_dtypes.bfloat16)
    lo = (x - hi.astype(np.float32)).astype(ml_dtypes.bfloat16)
    return hi, lo


def _build(reps=1):
    import concourse.bass as bass
    import concourse.tile as tile
    from concourse import bacc, mybir
    from contextlib import ExitStack

    f32 = mybir.dt.float32
    bf16 = mybir.dt.bfloat16
    AF = mybir.ActivationFunctionType

    nc = bacc.Bacc("TRN2", target_bir_lowering=False, debug=False,
                   enable_asserts=True, num_devices=NCORES)

    zhi_d = nc.dram_tensor("z_hi", [128, QUARTER], bf16, kind="ExternalInput").ap()
    zlo_d = nc.dram_tensor("z_lo", [128, QUARTER], bf16, kind="ExternalInput").ap()
    whi_d = nc.dram_tensor("wtT_hi", [128, WIDTH], bf16, kind="ExternalInput").ap()
    wlo_d = nc.dram_tensor("wtT_lo", [128, WIDTH], bf16, kind="ExternalInput").ap()
    l2_d = nc.dram_tensor("lhsT2", [128, 128], bf16, kind="ExternalInput").ap()
    lg_d = nc.dram_tensor("lhsTg", [128, 2], bf16, kind="ExternalInput").ap()
    b_d = nc.dram_tensor("bias", [128, 1], f32, kind="ExternalInput").ap()

    dzT_d = nc.dram_tensor("dzT", [128, QUARTER], f32, kind="ExternalOutput").ap()
    dlp_d = nc.dram_tensor("dlp", [4, QUARTER], f32, kind="ExternalOutput").ap()

    with tile.TileContext(nc) as tc, ExitStack() as ctx:
        const = ctx.enter_context(tc.tile_pool(name="const", bufs=1))
        zin = ctx.enter_context(tc.tile_pool(name="zin", bufs=2))
        shp = ctx.enter_context(tc.tile_pool(name="sh", bufs=2))
        sgp = ctx.enter_context(tc.tile_pool(name="sg", bufs=2))
        dzp = ctx.enter_context(tc.tile_pool(name="dz", bufs=2))
        dlpp = ctx.enter_context(tc.tile_pool(name="dlp", bufs=2))
        # PSUM: p1 2 banks x2 + p2 1 bank x2 + pg 1 bank x2 = 8 banks
        p1p = ctx.enter_context(tc.tile_pool(name="p1", bufs=2, space="PSUM"))
        p2p = ctx.enter_context(tc.tile_pool(name="p2", bufs=2, space="PSUM"))
        pgp = ctx.enter_context(tc.tile_pool(name="pg", bufs=2, space="PSUM"))

        whi_t = const.tile([128, WIDTH], bf16)
        nc.sync.dma_start(whi_t[:], whi_d[:])
        wlo_t = const.tile([128, WIDTH], bf16)
        nc.sync.dma_start(wlo_t[:], wlo_d[:])
        l2_t = const.tile([128, 128], bf16)
        nc.sync.dma_start(l2_t[:], l2_d[:])
        lg_t = const.tile([128, 2], bf16)
        nc.sync.dma_start(lg_t[:], lg_d[:])
        b_t = const.tile([128, 1], f32)
        nc.sync.dma_start(b_t[:], b_d[:])

        zhi_t = zlo_t = None
        dzbig = None
        pg = None
        dlpacc = None

        def body():
            for m in range(NSB):
                emit_sb(m)

        def emit_sb(m):
            nonlocal zhi_t, zlo_t, dzbig, pg, dlpacc
            zn = ZCHUNK // SB       # sbs per z chunk
            if m % zn == 0:
                zhi_t = zin.tile([128, ZCHUNK], bf16, tag="zhi")
                nc.sync.dma_start(zhi_t[:], zhi_d[:, m * SB:m * SB + ZCHUNK])
                zlo_t = zin.tile([128, ZCHUNK], bf16, tag="zlo")
                nc.sync.dma_start(zlo_t[:], zlo_d[:, m * SB:m * SB + ZCHUNK])
            c4 = (m % zn) * SB
            zhc = zhi_t[:, c4:c4 + SB]
            zlc = zlo_t[:, c4:c4 + SB]

            # mm1: 3 hi/lo passes x 4 concurrent row/col tiles
            p1 = p1p.tile([128, 2 * SB], f32)
            for ip, (wt, zt) in enumerate(((whi_t, zhc), (whi_t, zlc),
                                           (wlo_t, zhc))):
                for q in range(4):
                    half, bank = q % 2, q // 2
                    nc.tensor.matmul(
                        p1[half * 64:(half + 1) * 64, bank * SB:(bank + 1) * SB],
                        wt[32 * q:32 * (q + 1), :],
                        zt[32 * q:32 * (q + 1), :],
                        start=(ip == 0), stop=(ip == 2),
                        tile_position=(32 * q, 64 * half),
                    )

            sh = shp.tile([128, 2 * SB], bf16)
            nc.scalar.activation(sh[:], p1[:], AF.Tanh, bias=b_t[:], scale=1.0)
            sg = sgp.tile([128, 2 * SB], bf16)
            nc.vector.tensor_mul(sg[:], sh[:], sh[:])

            # mm2h: dz.T packed (partition 32q+d)
            p2 = p2p.tile([128, SB], f32)
            nc.tensor.matmul(p2[0:64, :], l2_t[:, 0:64], sh[:, 0:SB],
                             start=True, stop=True, tile_position=(0, 0))
            nc.tensor.matmul(p2[64:128, :], l2_t[:, 64:128], sh[:, SB:2 * SB],
                             start=True, stop=True, tile_position=(0, 64))

            dn = DZCHUNK // SB
            if m % dn == 0:
                dzbig = dzp.tile([128, DZCHUNK], f32)
            cd = (m % dn) * SB
            # split the dz evacuation between ACT and DVE by columns
            nc.scalar.activation(dzbig[:, cd:cd + ACTCOLS], p2[:, 0:ACTCOLS],
                                 AF.Copy)
            nc.vector.tensor_copy(dzbig[:, cd + ACTCOLS:cd + SB],
                                  p2[:, ACTCOLS:SB])
            if m % dn == dn - 1:
                nc.sync.dma_start(dzT_d[:, (m - dn + 1) * SB:(m + 1) * SB],
                                  dzbig[:])

            # mm2g: dlogp strips; partition offset rotates over sb pairs
            off = 64 * (m % 2)
            if m % 2 == 0:
                pg = pgp.tile([128, SB], f32)
            nc.tensor.matmul(pg[off + 0:off + 2, :], lg_t[:], sg[:, 0:SB],
                             start=True, stop=True, tile_position=(0, off))
            nc.tensor.matmul(pg[off + 32:off + 34, :], lg_t[:], sg[:, SB:2 * SB],
                             start=True, stop=True, tile_position=(0, off + 32))
            if m % DLPGRP == 0:
                dlpacc = dlpp.tile([128, DLPGRP * SB // 2], f32)
            if m % 2 == 1:
                k = (m % DLPGRP) // 2
                nc.vector.tensor_copy(dlpacc[0:98, k * SB:(k + 1) * SB],
                                      pg[0:98, :])
            if m % DLPGRP == DLPGRP - 1:
                base = m - DLPGRP + 1
                np_ = DLPGRP // 2      # sb pairs in this flush group
                # dlpacc[{0,1},   k*SB+n] = dlogp q0/q1, sb base+2k (even)
                # dlpacc[{32,33}, k*SB+n] = dlogp q2/q3, sb base+2k
                # dlpacc[{64,65}, k*SB+n] = dlogp q0/q1, sb base+2k+1 (odd)
                # dlpacc[{96,97}, k*SB+n] = dlogp q2/q3, sb base+2k+1
                for (p0, r0, o0) in ((0, 0, 0), (32, 2, 0), (64, 0, 1),
                                     (96, 2, 1)):
                    src = dlpacc[p0:p0 + 2, :].rearrange(
                        "p (k n) -> p k n", n=SB)
                    dst_ap = dlp_d[r0:r0 + 2, base * SB:(base + DLPGRP) * SB] \
                        .rearrange("p (k two n) -> p two k n", two=2, n=SB)[:, o0]
                    nc.sync.dma_start(dst_ap, src)

        if reps == 1:
            body()
        else:
            # timing-only variant: hardware loop repeating identical work
            with tc.For_i(0, reps, 1):
                body()

    nc.compile()
    return nc


def _get_nc(reps=1):
    key = f"nc{reps}"
    if key not in _CACHE:
        _CACHE[key] = _build(reps)
    return _CACHE[key]


def _prep_inputs(t, z,
                 W_centres, W_log_sigmas, W_lin_w, W_lin_b,
                 U_centres, U_log_sigmas, U_lin_w, U_lin_b,
                 B_centres, B_log_sigmas, B_lin_w, B_lin_b):
    t = np.asarray(t, np.float32)
    Wt = _hypernet(t, np.asarray(W_centres, np.float32),
                   np.asarray(W_log_sigmas, np.float32),
                   np.asarray(W_lin_w, np.float32),
                   np.asarray(W_lin_b, np.float32)).reshape(WIDTH, D)
    Ut = _hypernet(t, np.asarray(U_centres, np.float32),
                   np.asarray(U_log_sigmas, np.float32),
                   np.asarray(U_lin_w, np.float32),
                   np.asarray(U_lin_b, np.float32)).reshape(WIDTH, D)
    Bt = _hypernet(t, np.asarray(B_centres, np.float32),
                   np.asarray(B_log_sigmas, np.float32),
                   np.asarray(B_lin_w, np.float32),
                   np.asarray(B_lin_b, np.float32))
    wu = np.sum(Wt * Ut, axis=1)

    U64 = (Ut / np.float32(WIDTH)).astype(np.float32)          # (64, 32)
    wu64 = (wu / np.float32(WIDTH)).astype(np.float32)         # (64,)

    import ml_dtypes
    bf = ml_dtypes.bfloat16

    wtT = np.tile(Wt.T, (4, 1)).astype(np.float32)             # (128, 64)
    wtT_hi, wtT_lo = _split_bf16(wtT)

    blk = np.zeros((128, 64), np.float32)
    blk[0:64, 0:32] = U64
    blk[64:128, 32:64] = U64
    lhsT2 = np.concatenate([blk, blk], axis=1).astype(bf)      # (128, 128)
    lhsTg = np.zeros((128, 2), np.float32)
    lhsTg[0:64, 0] = wu64
    lhsTg[64:128, 1] = wu64
    lhsTg = lhsTg.astype(bf)
    bias = np.concatenate([Bt, Bt]).reshape(128, 1).astype(np.float32)

    z = np.ascontiguousarray(np.asarray(z, np.float32))
    # per-core packed transpose: (8, 4, 32768, 32) -> (8, 4, 32, 32768)
    zT = z.reshape(NCORES, 4, QUARTER, D).transpose(0, 1, 3, 2) \
          .reshape(NCORES, 128, QUARTER)
    z_hi, z_lo = _split_bf16(zT)
    z_hi = np.ascontiguousarray(z_hi)
    z_lo = np.ascontiguousarray(z_lo)

    in_maps = [
        dict(z_hi=z_hi[c], z_lo=z_lo[c], wtT_hi=wtT_hi, wtT_lo=wtT_lo,
             lhsT2=lhsT2, lhsTg=lhsTg, bias=bias)
        for c in range(NCORES)
    ]
    return in_maps, wu


def _postprocess(results, wu):
    const = np.float32(np.sum(wu) / np.float32(WIDTH))
    dz_parts, dlp_parts = [], []
    for c in range(NCORES):
        dzT = results[c]["dzT"]                       # (128, 32768)
        dz = dzT.reshape(4, D, QUARTER).transpose(0, 2, 1).reshape(PER_CORE, D)
        dz_parts.append(dz)
        dlp = results[c]["dlp"].reshape(PER_CORE) - const
        dlp_parts.append(dlp)
    dz_dt = np.ascontiguousarray(np.concatenate(dz_parts, axis=0), dtype=np.float32)
    dlogp = np.concatenate(dlp_parts, axis=0).reshape(BATCH, 1).astype(np.float32)
    return dz_dt, dlogp


def kernel(t, z, logp_z, **params):
    from concourse.bass_utils import run_bass_kernel_spmd

    nc = _get_nc()
    in_maps, wu = _prep_inputs(t, z, **params)
    res = run_bass_kernel_spmd(nc, in_maps, core_ids=list(range(NCORES)))
    return _postprocess(res.results, wu)
